# revision 8
# baseline (speedup 1.0000x reference)
"""CRNN (conv3x3 -> ReLU -> freq-maxpool -> GRU scan -> FC) on 8 Trainium2
NeuronCores, data-parallel over batch (8 items per core).

Structure per core:
  - conv: banded-weight matmuls over the frequency contraction; time shifts
    via column offsets into a padded fp32r tile; two accumulating matmuls per
    f-pair give PSUM [128 = 2f x 64c, 512t]; running tensor_max over f-pairs
    + ReLU(+bias) writes feat[c, t] batch-interleaved into bigU[64:128].
  - xn = W_ihn @ feat + b_ihn precomputed (PE), packed into bigH[64:128].
  - GRU scan with u/v decomposition: h_{k+1} = u_k + v_k, u_k = z_k*h_k,
    v_k = (1-z_k)*n_k. The rz matmul takes [u; feat] (K=128) plus a separate
    v matmul (K=64), so the only late operand on the serial chain is v.
  - FC from bigH h-history, output DMA'd straight from PSUM.
  - The time-half-1 conv work, the second half of xn, and the FC tiles are
    emitted interleaved with the scan steps so they execute in the scan's
    idle engine slots.
"""

import contextlib
import numpy as np

import concourse.bass as bass
import concourse.mybir as mybir
import concourse.tile as tile
from concourse import bacc
from concourse.bass_utils import run_bass_kernel_spmd

F32 = mybir.dt.float32
F32R = mybir.dt.float32r
AF = mybir.ActivationFunctionType
OP = mybir.AluOpType

B, F, T = 64, 64, 1024
C = 64
H = 64
OUT = 2
NCORES = 8
NB = B // NCORES
NFP = F // 2


def build_crnn(nb=NB, t_steps=T, reps=1, phases=("conv", "xn", "scan", "fc"),
               interleave=False):
    # interleave=False (phase-sequential) measures ~2.94 ms/iter on HW vs
    # ~3.0-3.4 interleaved: injecting conv/xn/fc work into the scan's slots
    # disrupts the latency-critical GRU chain more than it hides (the
    # opposite of what the cost-model simulator predicts).
    nc = bacc.Bacc("TRN2", target_bir_lowering=False, debug=False)
    TB = t_steps * nb
    NTH = max(1, t_steps // 512)
    THW = min(512, t_steps)
    NJ = max(1, TB // 512)
    JW = min(512, TB)
    full = len(phases) == 4
    inter = interleave and full and t_steps == T

    x_d = nc.declare_dram_parameter("x", [nb, F, t_steps], F32, isOutput=False)
    convA_d = nc.declare_dram_parameter("convA", [128, NFP * 128], F32, isOutput=False)
    convB_d = nc.declare_dram_parameter("convB", [64, NFP * 128], F32, isOutput=False)
    cb_d = nc.declare_dram_parameter("conv_bias", [C, 1], F32, isOutput=False)
    wrz_d = nc.declare_dram_parameter("w_rz_lhsT", [128, 128], F32, isOutput=False)
    wn_d = nc.declare_dram_parameter("w_n_lhsT", [H, H], F32, isOutput=False)
    win_d = nc.declare_dram_parameter("w_in_lhsT", [C, H], F32, isOutput=False)
    brz_d = nc.declare_dram_parameter("b_rz", [128, 1], F32, isOutput=False)
    brzn_d = nc.declare_dram_parameter("b_rz_neg", [H, 1], F32, isOutput=False)
    bhn_d = nc.declare_dram_parameter("b_hn", [H, 1], F32, isOutput=False)
    bin_d = nc.declare_dram_parameter("b_in_row", [1, H], F32, isOutput=False)
    fcw_d = nc.declare_dram_parameter("fc_lhsT", [H, OUT], F32, isOutput=False)
    fcb_d = nc.declare_dram_parameter("fc_b_row", [1, OUT], F32, isOutput=False)
    out_d = nc.declare_dram_parameter("out", [nb, OUT, t_steps], F32, isOutput=True)

    with tile.TileContext(nc) as tc:
        with (
            tc.tile_pool(name="persist", bufs=1) as persist,
            tc.tile_pool(name="stage", bufs=2) as stage,
            tc.tile_pool(name="x2pool", bufs=1) as x2p,
            tc.tile_pool(name="work", bufs=2) as work,
            tc.tile_pool(name="scanw", bufs=3) as scanw,
            tc.tile_pool(name="pp_conv", bufs=2, space="PSUM") as ppc,
            tc.tile_pool(name="pp_scan", bufs=2, space="PSUM") as pps,
            tc.tile_pool(name="pp_misc", bufs=2, space="PSUM") as ppm,
        ):
            convA = persist.tile([128, NFP * 128], F32R)
            convB = persist.tile([64, NFP * 128], F32R)
            cb = persist.tile([C, 1], F32)
            w_rz = persist.tile([128, 128], F32)
            w_n = persist.tile([H, H], F32)
            w_in_full = persist.tile([128, H], F32)
            w_in = w_in_full[64:128, :]
            b_rz = persist.tile([128, 1], F32)
            b_rz_neg = persist.tile([H, 1], F32)
            b_hn = persist.tile([H, 1], F32)
            b_in = persist.tile([1, H], F32)
            fc_w = persist.tile([H, OUT], F32)
            fc_b = persist.tile([1, OUT], F32)
            ones = persist.tile([1, JW], F32)
            # bigU: rows 0:64 = u_{k-1} at blk k, rows 64:128 = feat_k at blk k
            bigU = persist.tile([128, (t_steps + 1) * nb], F32)
            # bigH: rows 0:64 = h_k at blk k, rows 64:128 = xn_k at blk k
            bigH = persist.tile([128, (t_steps + 1) * nb], F32)
            v_zero = persist.tile([H, nb], F32)

            CW = NFP * 128 // 4
            for ci in range(4):
                cs = slice(ci * CW, (ci + 1) * CW)
                stg = stage.tile([128, CW], F32, tag="stg", name="stg")
                nc.sync.dma_start(out=stg, in_=convA_d[:, cs])
                nc.vector.tensor_copy(convA[:, cs], stg)
            for ci in range(4):
                cs = slice(ci * CW, (ci + 1) * CW)
                stg = stage.tile([128, CW], F32, tag="stg", name="stgb")
                nc.sync.dma_start(out=stg[0:64, :], in_=convB_d[:, cs])
                nc.vector.tensor_copy(convB[:, cs], stg[0:64, :])

            nc.sync.dma_start(out=cb, in_=cb_d[:, :])
            nc.sync.dma_start(out=w_rz, in_=wrz_d[:, :])
            nc.sync.dma_start(out=w_n, in_=wn_d[:, :])
            nc.sync.dma_start(out=w_in, in_=win_d[:, :])
            nc.sync.dma_start(out=b_rz, in_=brz_d[:, :])
            nc.sync.dma_start(out=b_rz_neg, in_=brzn_d[:, :])
            nc.sync.dma_start(out=b_hn, in_=bhn_d[:, :])
            nc.sync.dma_start(out=b_in, in_=bin_d[:, :])
            nc.sync.dma_start(out=fc_w, in_=fcw_d[:, :])
            nc.sync.dma_start(out=fc_b, in_=fcb_d[:, :])
            nc.vector.memset(ones, 1.0)
            nc.vector.memset(bigU[0:64, 0:nb], 0.0)   # u_{-1} = 0
            nc.vector.memset(bigH[0:64, 0:nb], 0.0)   # h_0 = 0
            nc.vector.memset(v_zero, 0.0)             # v_{-1} = 0
            if not full:
                nc.vector.memset(bigU[:, :], 0.0)
                nc.vector.memset(bigH[:, :], 0.0)

            # ---------- X2R staging (persistent, per batch) ----------
            X2Rs = []
            if "conv" in phases:
                for b in range(nb):
                    X2 = x2p.tile([128, t_steps + 2], F32, tag="x2", name="x2")
                    nc.sync.dma_start(out=X2[0:64, 1 : t_steps + 1], in_=x_d[b, :, :])
                    nc.sync.dma_start(out=X2[64:128, 0:t_steps], in_=x_d[b, :, :])
                    nc.vector.memset(X2[0:64, 0:1], 0.0)
                    nc.vector.memset(X2[0:64, t_steps + 1 : t_steps + 2], 0.0)
                    nc.vector.memset(X2[64:128, t_steps : t_steps + 2], 0.0)
                    X2R = persist.tile([128, t_steps + 2], F32R, name=f"x2r{b}")
                    nc.vector.tensor_copy(X2R, X2)
                    X2Rs.append(X2R)

            # ---------- emission units ----------
            conv_state = {}

            def conv_mm(b, th, fp):
                ps = ppc.tile([128, THW], F32, tag="cps", name="cps")
                X2R = X2Rs[b]
                nc.tensor.matmul(
                    ps, convA[:, fp * 128 : (fp + 1) * 128],
                    X2R[:, th * THW : th * THW + THW],
                    start=True, stop=False,
                )
                nc.tensor.matmul(
                    ps, convB[:, fp * 128 : (fp + 1) * 128],
                    X2R[0:64, th * THW + 2 : th * THW + THW + 2],
                    start=False, stop=True,
                )
                if fp == 0:
                    macc = work.tile([128, THW], F32, tag="macc", name="macc")
                    conv_state[(b, th)] = macc
                    nc.vector.tensor_copy(macc, ps)
                else:
                    nc.vector.tensor_max(conv_state[(b, th)],
                                         conv_state[(b, th)], ps)

            def conv_tail(b, th):
                macc = conv_state.pop((b, th))
                mhi = work.tile([64, THW], F32, tag="mhi", name="mhi")
                nc.vector.tensor_copy(mhi, macc[64:128, :])
                m2 = work.tile([64, THW], F32, tag="m2", name="m2")
                nc.vector.tensor_max(m2, macc[0:64, :], mhi)
                out_ap = bigU[64:128, th * THW * nb + b : (th * THW + THW) * nb : nb]
                nc.scalar.activation(out_ap, m2, AF.Relu, bias=cb)

            def xn_unit(j):
                ps = ppm.tile([H, JW], F32, tag="mps", name="xnps")
                nc.tensor.matmul(
                    ps, w_in, bigU[64:128, j * JW : (j + 1) * JW],
                    start=True, stop=False,
                )
                nc.tensor.matmul(ps, b_in, ones, start=False, stop=True)
                nc.scalar.copy(bigH[64:128, j * JW : (j + 1) * JW], ps)

            def fc_unit(j):
                ps = ppm.tile([OUT, JW], F32, tag="mps", name="fcps")
                nc.tensor.matmul(
                    ps, fc_w, bigH[0:64, nb + j * JW : nb + (j + 1) * JW],
                    start=True, stop=False,
                )
                nc.tensor.matmul(ps, fc_b, ones, start=False, stop=True)
                ob = work.tile([OUT, JW], F32, tag="ob", name="ob")
                nc.scalar.copy(ob, ps)
                tpj = JW // nb
                for b in range(nb):
                    nc.sync.dma_start(
                        out=out_d[b, 0:OUT, j * tpj : (j + 1) * tpj],
                        in_=ob[:, b : JW : nb],
                    )

            def scan_step(k, prev_v):
                col = slice(k * nb, (k + 1) * nb)
                ncol = slice((k + 1) * nb, (k + 2) * nb)
                psum_rz = pps.tile([128, nb], F32, tag="rz", name="rz")
                psum_hn = pps.tile([H, nb], F32, tag="hn", name="hn")
                nc.tensor.matmul(psum_rz, w_rz, bigU[:, col], start=True, stop=False)
                nc.tensor.matmul(psum_hn, w_n, bigH[0:64, col], start=True, stop=True)
                nc.tensor.matmul(psum_rz, w_rz[0:64, :], prev_v, start=False, stop=True)

                r_s = scanw.tile([H, nb], F32, tag="rs", name="rs")
                nc.scalar.activation(r_s, psum_rz[0:64, :], AF.Sigmoid,
                                     bias=b_rz[0:64, :])
                z_s = scanw.tile([H, nb], F32, tag="zs", name="zs")
                nc.scalar.activation(z_s, psum_rz[64:128, :], AF.Sigmoid,
                                     bias=b_rz[64:128, :])
                zb_s = scanw.tile([H, nb], F32, tag="zbs", name="zbs")
                nc.scalar.activation(zb_s, psum_rz[64:128, :], AF.Sigmoid,
                                     bias=b_rz_neg, scale=-1.0)
                nc.vector.tensor_mul(bigU[0:64, ncol], z_s, bigH[0:64, col])
                q = scanw.tile([128, nb], F32, tag="q", name="q")
                nc.vector.scalar_tensor_tensor(
                    out=q[64:128, :], in0=psum_hn, scalar=b_hn, in1=r_s,
                    op0=OP.add, op1=OP.mult,
                )
                q2 = scanw.tile([H, nb], F32, tag="q2", name="q2")
                nc.vector.tensor_add(q2, q[64:128, :], bigH[64:128, col])
                n_t = scanw.tile([H, nb], F32, tag="n", name="n")
                nc.scalar.activation(n_t, q2, AF.Tanh)
                v_t = scanw.tile([H, nb], F32, tag="v", name="v")
                nc.vector.tensor_mul(v_t, zb_s, n_t)
                nc.vector.tensor_add(bigH[0:64, ncol], bigU[0:64, ncol], v_t)
                return v_t

            rep_ctx = tc.For_i(0, reps, 1) if reps > 1 else contextlib.nullcontext()
            with rep_ctx:
                if not inter:
                    for b in range(nb if "conv" in phases else 0):
                        for th in range(NTH):
                            for fp in range(NFP):
                                conv_mm(b, th, fp)
                            conv_tail(b, th)
                    for j in range(NJ if "xn" in phases else 0):
                        xn_unit(j)
                    prev_v = v_zero
                    for k in range(t_steps if "scan" in phases else 0):
                        prev_v = scan_step(k, prev_v)
                    for j in range(NJ if "fc" in phases else 0):
                        fc_unit(j)
                else:
                    # th=0 conv upfront + first-half xn
                    for b in range(nb):
                        for fp in range(NFP):
                            conv_mm(b, 0, fp)
                        conv_tail(b, 0)
                    for j in range(NJ // 2):
                        xn_unit(j)

                    # conv th=1 spread over scan steps [8, 440); 2nd-half xn
                    # after it; each fc tile as soon as its h-range is done.
                    units = []
                    for b in range(nb):
                        for fp in range(NFP):
                            units.append(("mm", b, fp))
                        units.append(("tail", b))
                    sched = {}
                    lo, hi = 8, 440
                    for i, u in enumerate(units):
                        k_at = lo + (i * (hi - lo)) // len(units)
                        sched.setdefault(k_at, []).append(u)
                    for j in range(NJ // 2, NJ):
                        sched.setdefault(444 + 8 * (j - NJ // 2), []).append(("xn", j))
                    tpj = JW // nb
                    for j in range(NJ):
                        k_at = (j + 1) * tpj
                        if k_at < t_steps:
                            sched.setdefault(k_at, []).append(("fc", j))

                    prev_v = v_zero
                    for k in range(t_steps):
                        prev_v = scan_step(k, prev_v)
                        for u in sched.get(k, ()):
                            if u[0] == "mm":
                                conv_mm(u[1], 1, u[2])
                            elif u[0] == "tail":
                                conv_tail(u[1], 1)
                            elif u[0] == "xn":
                                xn_unit(u[1])
                            elif u[0] == "fc":
                                fc_unit(u[1])
                    for j in range(NJ):
                        if (j + 1) * tpj >= t_steps:
                            fc_unit(j)

    nc.finalize()
    return nc


def prep_weights(conv_w, conv_b, w_ih, w_hh, b_ih, b_hh, fc_w, fc_b):
    """Host-side rearrangement of the small weights into device layouts."""
    conv_w = np.asarray(conv_w, np.float32)
    A = np.zeros((128, NFP * 128), np.float32)
    Bm = np.zeros((64, NFP * 128), np.float32)
    for fp in range(NFP):
        for fo in range(2):
            fout = 2 * fp + fo
            for fprime in range(max(0, fout - 1), min(64, fout + 2)):
                i = fprime - fout + 1
                cols = slice(fp * 128 + fo * 64, fp * 128 + fo * 64 + 64)
                A[fprime, cols] = conv_w[:, 0, i, 0]
                A[64 + fprime, cols] = conv_w[:, 0, i, 1]
                Bm[fprime, cols] = conv_w[:, 0, i, 2]
    w_ih = np.asarray(w_ih, np.float32)
    w_hh = np.asarray(w_hh, np.float32)
    b_ih = np.asarray(b_ih, np.float32)
    b_hh = np.asarray(b_hh, np.float32)
    return {
        "convA": A,
        "convB": Bm,
        "conv_bias": np.asarray(conv_b, np.float32).reshape(C, 1),
        "w_rz_lhsT": np.concatenate(
            [w_hh[0:128, :].T, w_ih[0:128, :].T], axis=0
        ).astype(np.float32).copy(),
        "w_n_lhsT": w_hh[128:192, :].T.astype(np.float32).copy(),
        "w_in_lhsT": w_ih[128:192, :].T.astype(np.float32).copy(),
        "b_rz": (b_ih[0:128] + b_hh[0:128]).reshape(128, 1).astype(np.float32),
        "b_rz_neg": (-(b_ih[64:128] + b_hh[64:128])).reshape(H, 1).astype(np.float32),
        "b_hn": b_hh[128:192].reshape(H, 1).astype(np.float32),
        "b_in_row": b_ih[128:192].reshape(1, H).astype(np.float32),
        "fc_lhsT": np.asarray(fc_w, np.float32).T.copy(),
        "fc_b_row": np.asarray(fc_b, np.float32).reshape(1, OUT),
    }


_NC_CACHE = {}


def _get_nc():
    if "nc" not in _NC_CACHE:
        _NC_CACHE["nc"] = build_crnn()
    return _NC_CACHE["nc"]


def run(inputs, trace=False):
    """Returns (out [B, OUT, T], BassKernelResults)."""
    x = np.asarray(inputs["x"], np.float32)
    wd = prep_weights(
        inputs["conv_w"], inputs["conv_b"], inputs["w_ih"], inputs["w_hh"],
        inputs["b_ih"], inputs["b_hh"], inputs["fc_w"], inputs["fc_b"],
    )
    nc = _get_nc()
    in_maps = []
    for i in range(NCORES):
        m = dict(wd)
        m["x"] = np.ascontiguousarray(x[i * NB : (i + 1) * NB])
        in_maps.append(m)
    res = run_bass_kernel_spmd(nc, in_maps, list(range(NCORES)), trace=trace)
    out = np.concatenate([res.results[i]["out"] for i in range(NCORES)], axis=0)
    return out, res


def kernel(**inputs) -> np.ndarray:
    out, _ = run(inputs, trace=False)
    return out

